# revision 49
# baseline (speedup 1.0000x reference)
"""Causal single-head attention on 8 Trainium2 NeuronCores.

Problem: B=4, S=2048, D_IN=1024, D_OUT=64 (fp32).
  Q = Xq @ Wq; K = Xk @ Wk; V = Xv @ Wv
  out = softmax(mask(Q K^T / 8)) @ V

Sharding: 8 cores = 4 batches x 2 interleaved query-block sets.
Core c handles batch b = c//2 and query blocks {128*(2t+h) : t in 0..7},
h = c%2.  The interleave balances causal work between the pair.

v8 design (vs the v1 baseline: ~238 engine instructions/call vs ~800;
HBM traffic 6 MB/core vs 12 MB; 96 matmuls vs ~288):
  * X is cast to bf16 AND pre-transposed on the host into the exact
    SBUF tile layout [128, slot, 8, 512] = [d%128, s-group, d-tile,
    s%512], with the K, V and Q slices packed into ONE tensor loaded by
    ONE DMA; the 1/sqrt(E) attention scale is folded into Wq on the
    host.
  * One exchange collective instead of two: the K^T and V^T half
    projections live in one [64, 2, 1024] buffer and are
    pair-AllGathered in a single shot.
  * V1 = [V | 1] is built by TWO multi-tile xbar DMA-transposes from
    the gathered DRAM buffer plus ONE tracked single-tile guard
    re-write (the multi-tile form escapes the tile dep-tracker; HWDGE
    FIFO order makes the guard's completion imply both multis').
  * Scores stay transposed (keys on partitions) and are computed
    full-width into a 4-bank PSUM quad; exp runs once per quad.  A
    host-baked full causal mask (one DVE multiply per q-half) zeroes
    every dead column, so AV needs no sub-ranges.
  * AV^T accumulates a two-bank [65, 2, 512] PSUM tile (one bank per
    q-half); row 64 (from the ones column of V1) is the softmax
    denominator.  One copy + one plain DMA ship the fp32 blocks to the
    host, which does the final O(S*E) divide + transpose in numpy.

All loop structure is SPMD-uniform; h enters only via host-side data.
"""

import os
import numpy as np
import ml_dtypes

import concourse.bass as bass
import concourse.mybir as mybir
import concourse.tile as tile
from concourse.bass_utils import run_bass_kernel_spmd
from concourse.masks import make_identity
from concourse.vector_clock import ScopedClock

# ---------------------------------------------------------------------------
# Workaround: the walrus in this container rejects Tile's end-of-kernel drain
# when it carries >1 sem wait ("Too many sync wait commands").  Split the
# waits across single-wait SP NOPs placed just before the drain.
# ---------------------------------------------------------------------------


def _patched_drain_and_barrier(self, tick_clock, wait_clock):
    nc = self.nc
    collector = nc.sync.nop(nofuse=True)
    wait_clock.add_sem_waits(
        collector.ins, ScopedClock({None: tick_clock.global_clock})
    )
    si = collector.ins.sync_info
    waits = list(si.on_wait or []) if si is not None else []
    if si is not None:
        si.on_wait = waits[:1]
    for w in waits[1:]:
        n = nc.sync.nop(nofuse=True)
        nsi = n.ins.sync_info
        if nsi is None:
            n.ins.sync_info = mybir.SyncInfo(on_wait=[w], on_update=[])
        else:
            nsi.on_wait = [w]
    nc.sync.drain()
    nc.all_engine_barrier()
    assert self.sems is not None
    popped = nc._tile_sem_poison_stack.pop()
    assert popped is self._sem_poison
    nc.clear_and_free_semaphores(list(self.sems.allocated().values()))
    nc.all_engine_barrier()


tile.TileContext._drain_and_barrier = _patched_drain_and_barrier


def _split_sync_waits(nc, limit=1):
    """The nix walrus allows only `limit` sem waits per instruction; hoist
    extras onto same-engine NOPs placed immediately before the instruction."""
    ctr = [0]
    for fn in nc.m.functions:
        for bb in fn.blocks:
            out_list = []
            changed = False
            for inst in bb.instructions:
                si = inst.sync_info
                waits = list(si.on_wait) if si is not None and si.on_wait else []
                if len(waits) > limit:
                    keep = waits[-limit:]
                    for w in waits[:-limit]:
                        ctr[0] += 1
                        nop = mybir.InstNoOp(
                            name=f"waitsplit-{ctr[0]}",
                            engine=inst.engine,
                            ins=[],
                            outs=[],
                            sync_info=mybir.SyncInfo(on_wait=[w], on_update=[]),
                        )
                        out_list.append(nop)
                    si.on_wait = keep
                    changed = True
                out_list.append(inst)
            if changed:
                bb.instructions = out_list

# ---------------------------------------------------------------------------

B, S, D, E = 4, 2048, 1024, 64
SC = S // 2          # query rows per core
NT = SC // 128       # 8 local query blocks
NKT = S // 128       # 16 k-tiles
ND = D // 128        # 8 d-tiles
GROUP = 512          # s columns per projection group
NG = SC // GROUP     # 2 groups per tensor

F32 = mybir.dt.float32
BF16 = mybir.dt.bfloat16
EXP = mybir.ActivationFunctionType.Exp

# Unique-signature tag: the jax/neuron compile cache keys collide for
# same-signature modules, so every kernel variant carries a dummy input
# whose shape encodes the variant id.
KERNEL_UID = 129


def _build_nc(
    loop_reps=None, timing_mode=False, use_cc=True, v1_dma=True, uid=KERNEL_UID
):
    nc = bass.Bass()

    # K/V source rows per core: half when the pair exchanges projections
    # via AllGather, full otherwise.
    ng_kv = NG if use_cc else 2 * NG

    # xkv packs K, V, then Q source slices along dim 1.
    # (Internal device-zeroed tensors in timing mode: shrinks the per-call
    # transfer so the K-rep wall-clock slope resolves true exec time.)
    nslot = 2 * ng_kv + NG
    if timing_mode:
        xkv = nc.dram_tensor("xkv", (128, nslot, ND, GROUP), BF16)
    else:
        xkv = nc.dram_tensor(
            "xkv", (128, nslot, ND, GROUP), BF16, kind="ExternalInput"
        )
    wq = nc.dram_tensor("wq", (128, ND, E), BF16, kind="ExternalInput")
    wk = nc.dram_tensor("wk", (128, ND, E), BF16, kind="ExternalInput")
    wv = nc.dram_tensor("wv", (128, ND, E), BF16, kind="ExternalInput")
    # Full causal 0/1 masks, baked per-core on the host:
    # masks[p, c, i, x] = 1 iff key (c*8+i)*128+p is visible to the query in
    # column x of q-half c (global row 128*(2*(4c + x//128) + h) + x%128).
    # Only k-tiles 8c..8c+7 of a half need masking (earlier k-tiles are
    # fully live), so ONE multiply per q-half fixes up everything.
    masks = nc.dram_tensor("masks", (128, 2, 8, GROUP), BF16, kind="ExternalInput")
    nc.dram_tensor("vtag", (1, uid), F32, kind="ExternalInput")
    # AV^T output: avh[c] = [V1^T exp^T](q-half c), rows 0:64 = V-weighted
    # sums, row 64 = softmax denominator.  Host divides + transposes.
    avh = nc.dram_tensor("avh", (E + 1, 2, GROUP), F32, kind="ExternalOutput")

    with tile.TileContext(nc) as tc:
        with (
            tc.tile_pool(name="const", bufs=1) as cpool,
            tc.tile_pool(name="fin", bufs=2) as fpool,
            tc.tile_pool(name="ps_proj", bufs=1, space="PSUM") as ps_proj,
            tc.tile_pool(name="ps_sc", bufs=1, space="PSUM") as ps_sc,
            tc.tile_pool(name="ps_av", bufs=1, space="PSUM") as ps_av,
            tc.tile_pool(name="ps_tp", bufs=2, space="PSUM") as ps_tp,
            tc.tile_pool(name="dram", bufs=1, space="DRAM") as dpool,
        ):
            # ---- one-time constants ----
            if not use_cc or not v1_dma:
                ident_b = cpool.tile([128, 128], BF16, tag="ident_b")
                make_identity(nc, ident_b)

            w_sb = {}
            for name, w in (("q", wq), ("k", wk), ("v", wv)):
                t = cpool.tile([128, ND, E], BF16, tag=f"w_{name}", name=f"w_{name}")
                nc.sync.dma_start(out=t[:], in_=w[:, :, :])
                w_sb[name] = t

            mask_sb = cpool.tile([128, 2, 8, GROUP], BF16, tag="masks")
            nc.sync.dma_start(out=mask_sb[:], in_=masks[:, :, :])

            # V1 = [V | 1]: inner dim padded to 80 (160 B) so each k-tile
            # slot is 32-byte aligned for the xbar DMA transpose.  The ones
            # column (the softmax-denominator trick) is set once; the per-call
            # body only rewrites cols 0:64.
            v1 = cpool.tile([128, NKT, 80], BF16, tag="v1")
            nc.vector.memset(v1[:], 1.0)

            def emit_body():
                qt = cpool.tile([E, SC], BF16, tag="qt")
                kt = cpool.tile([E, S], BF16, tag="kt")
                if use_cc:
                    # K^T half in [:, 0, :], V^T half in [:, 1, :]
                    kvh = cpool.tile([E, 2, SC], BF16, tag="kvh")
                if not (use_cc and v1_dma):
                    vt = cpool.tile([E, S], BF16, tag="vt")
                # exp'd transposed scores, one slot per (q-half, k-tile):
                # slots 0:8 = half 0, 8:24 = half 1.  Columns are shifted so
                # slot position 0 is the causal-boundary q-block t0 = kti//2.
                exph = cpool.tile([128, 24, GROUP], BF16, tag="exph")

                # ---- ONE X load on the SP (HWDGE) queue ----
                xkv_sb = cpool.tile(
                    [128, nslot, ND, GROUP], BF16, tag="x_kv", name="x_kv"
                )
                nc.sync.dma_start(out=xkv_sb[:], in_=xkv[:])
                x_sb = {"k": xkv_sb}  # v = slots ng_kv:, q = slots 2*ng_kv:

                def proj_mm(name, pt, slot, g):
                    """8 contract-tiled matmuls of one 512-column group into
                    PSUM slot `slot` (xkv slot `g`)."""
                    xg, w = x_sb["k"], w_sb[name]
                    if name == "v":
                        g = ng_kv + g
                    elif name == "q":
                        g = 2 * ng_kv + g
                    for dt in range(ND):
                        nc.tensor.matmul(
                            pt[:, slot, :],
                            w[:, dt, :],
                            xg[:, g, dt, :],
                            start=(dt == 0),
                            stop=(dt == ND - 1),
                        )

                def proj_copy(pt, dst, sl, scale):
                    src_ap = pt[:, 0 : (sl.stop - sl.start) // GROUP, :]
                    dst_ap = dst[:, sl].rearrange("e (g s) -> e g s", s=GROUP)
                    if scale is None:
                        nc.scalar.copy(out=dst_ap, in_=src_ap)
                    else:
                        nc.scalar.mul(dst_ap, src_ap, scale)

                # ---- K/V projections ----
                if use_cc:
                    # half projections, then ONE pair exchange
                    for name, col in (("k", 0), ("v", 1)):
                        pt = ps_proj.tile(
                            [E, 2, GROUP], F32, tag="proj", name=f"p_{name}"
                        )
                        for g in range(NG):
                            proj_mm(name, pt, g, g)
                        proj_copy(pt, kvh[:, col, :], slice(0, SC), None)

                    src_d = dpool.tile([E, 2, SC], BF16, tag="cc_src")
                    dst_d = dpool.tile([2, E, 2, SC], BF16, tag="cc_dst")
                    nc.gpsimd.dma_start(out=src_d[:], in_=kvh[:])
                    nc.gpsimd.collective_compute(
                        "AllGather",
                        mybir.AluOpType.bypass,
                        replica_groups=[[0, 1], [2, 3], [4, 5], [6, 7]],
                        ins=[src_d[:]],
                        outs=[dst_d[:]],
                    )
                    nc.gpsimd.dma_start(
                        out=kt[:].rearrange("e (r s) -> e r s", r=2),
                        in_=dst_d[:, :, 0, :].rearrange("r e s -> e r s"),
                    )
                    if not v1_dma:
                        nc.gpsimd.dma_start(
                            out=vt[:].rearrange("e (r s) -> e r s", r=2),
                            in_=dst_d[:, :, 1, :].rearrange("r e s -> e r s"),
                        )
                else:
                    # every core projects the full K and V itself
                    for name, dst in (("k", kt), ("v", vt)):
                        for gp in range(NG):
                            pt = ps_proj.tile(
                                [E, 2, GROUP], F32, tag="proj", name=f"p_{name}"
                            )
                            for g in range(2):
                                proj_mm(name, pt, g, 2 * gp + g)
                            proj_copy(
                                pt, dst, slice(2 * gp * GROUP, (2 * gp + 2) * GROUP),
                                None,
                            )

                # ---- Q projection (scale 1/sqrt(E) folded into wq on
                # the host) ----
                qpt = ps_proj.tile([E, 2, GROUP], F32, tag="proj", name="p_q")
                for g in range(NG):
                    proj_mm("q", qpt, g, g)
                proj_copy(qpt, qt, slice(0, SC), None)

                # ---- V1[:, kti, 0:64] = V k-tiles, natural layout ----
                if v1_dma:
                    # TWO multi-tile xbar DMA-transposes straight from the
                    # gathered DRAM buffer: [64, 1024] -> [128, 8, 64] writes
                    # v1[p, r*8+t, e] = V^T[e, r*1024 + t*128 + p].  Their 3D
                    # strided output escapes the tile dependency tracker, so
                    # ONE TRACKED single-tile transpose (slot 0) re-writes its
                    # data AFTER them: HWDGE transfers complete in FIFO order
                    # per issuing engine, so the guard's completion implies
                    # both multis are done; the first AV matmul (k-tile 0)
                    # waits on the guard, and the in-order PE queue then
                    # covers every later v1 read.
                    for r in range(2):
                        nc.sync.dma_start(
                            out=v1[:, r * 8 : (r + 1) * 8, 0:E],
                            in_=dst_d[r, :, 1, :],
                            transpose=True,
                        )
                    nc.sync.dma_start(
                        out=v1[:, 0, 0:E],
                        in_=dst_d[0, :, 1, 0:128],
                        transpose=True,
                    )
                else:
                    nc.vector.memset(v1[:], 1.0)
                    for kti in range(NKT):
                        tps = ps_tp.tile([128, 128], BF16, tag="tp")
                        nc.tensor.transpose(
                            tps[:, 0:E],
                            vt[:, kti * 128 : (kti + 1) * 128],
                            ident_b[0:E, 0:E],
                        )
                        nc.vector.tensor_copy(out=v1[:, kti, 0:E], in_=tps[:, 0:E])

                # ---- attention on q-column half c (cols c*512..c*512+511,
                # local q-blocks 4c..4c+3) ----
                av = ps_av.tile([E + 1, 2, GROUP], F32, tag="av")

                def attention_half(c):
                    slot0 = 8 * c  # exph slot base for this half
                    kti_hi = min(NKT, 8 * c + 8)  # k-tiles 0..kti_hi-1
                    # scores FULL-width (dead columns cost cycles, not
                    # instructions, and the full causal mask zeroes them);
                    # exp runs QUAD-wide over a 4-bank PSUM tile
                    for kti in range(kti_hi):
                        if kti % 4 == 0:
                            sps = ps_sc.tile([128, 4, GROUP], F32, tag="sc")
                        nc.tensor.matmul(
                            sps[:, kti % 4, :],
                            kt[:, kti * 128 : (kti + 1) * 128],
                            qt[:, c * GROUP : (c + 1) * GROUP],
                            start=True,
                            stop=True,
                        )
                        if kti % 4 == 3:
                            nc.scalar.activation(
                                exph[:, slot0 + kti - 3 : slot0 + kti + 1, :],
                                sps[:, :, :],
                                EXP,
                            )
                    # ONE causal fixup for the whole half: k-tiles 8c..8c+7
                    # (the half's earlier k-tiles are fully live)
                    mslot = slot0 + 8 * c
                    nc.vector.tensor_mul(
                        exph[:, mslot : mslot + 8, :],
                        exph[:, mslot : mslot + 8, :],
                        mask_sb[:, c, :, :],
                    )
                    # AV^T accumulation; row 64 = softmax denominator
                    for kti in range(kti_hi):
                        nc.tensor.matmul(
                            av[:, c, :],
                            v1[:, kti, 0 : E + 1],
                            exph[:, slot0 + kti, :],
                            start=(kti == 0),
                            stop=(kti == kti_hi - 1),
                            skip_group_check=True,
                        )

                attention_half(0)
                attention_half(1)

                # ship AV^T (+ denominator row) to the host.  avh is laid
                # out [65, 2, 512] so both the copy and the DMA are plain
                # (no rearrange -- rearranged DMA APs race, see memory)
                avsb = fpool.tile([E + 1, 2, GROUP], F32, tag="avsb")
                nc.vector.tensor_copy(out=avsb[:], in_=av[:])
                nc.sync.dma_start(out=avh[:, :, :], in_=avsb[:])

            if timing_mode:
                zt = cpool.tile([128, ND, GROUP], BF16, tag="zt")
                nc.vector.memset(zt[:], 0.0)
                for g in range(nslot):
                    nc.sync.dma_start(out=xkv[:, g], in_=zt[:])

            for _rep in range(1 if loop_reps is None else loop_reps):
                emit_body()

    _split_sync_waits(nc)
    return nc


_CACHE = {}
USE_CC = True


def _get_nc():
    if "nc" not in _CACHE:
        _CACHE["nc"] = _build_nc(use_cc=USE_CC)
    return _CACHE["nc"]


def _host_masks(h):
    """[128, 2, 8, 512] full causal masks for interleave h (see _build_nc)."""
    p = np.arange(128)
    x = np.arange(GROUP)
    m = np.empty((128, 2, 8, GROUP), dtype=np.float32)
    for c in range(2):
        q_glob = (2 * (4 * c + x // 128) + h) * 128 + x % 128   # [512]
        for i in range(8):
            k_glob = (8 * c + i) * 128 + p                       # [128]
            m[:, c, i, :] = (k_glob[:, None] <= q_glob[None, :])
    return np.ascontiguousarray(m).astype(ml_dtypes.bfloat16)


def _prep_xt(x_rows_f32):
    """[n*512 s, 1024 d] fp32 -> bf16 [128, n, 8, 512] = [d%128, g, dt, s%512]."""
    ng = x_rows_f32.shape[0] // GROUP
    xb = x_rows_f32.astype(ml_dtypes.bfloat16)
    xb = xb.reshape(ng, GROUP, ND, 128)          # [g, s', dt, p]
    return np.ascontiguousarray(xb.transpose(3, 0, 2, 1))


def _prep_w(w_f32):
    """[1024, 64] fp32 -> bf16 [128, 8, 64] = [d%128, dt, e]."""
    wb = w_f32.astype(ml_dtypes.bfloat16)
    return np.ascontiguousarray(wb.reshape(ND, 128, E).transpose(1, 0, 2))


def kernel(**inputs):
    xq_full = np.asarray(inputs["inputs_for_queries"], dtype=np.float32)
    xk_full = np.asarray(inputs["inputs_for_keys"], dtype=np.float32)
    xv_full = np.asarray(inputs["inputs_for_values"], dtype=np.float32)
    # 1/sqrt(E) attention scale folded into the Q weights
    wq = _prep_w(np.asarray(inputs["Weight_Q"], dtype=np.float32) / np.sqrt(E))
    wk = _prep_w(np.asarray(inputs["Weight_K"], dtype=np.float32))
    wv = _prep_w(np.asarray(inputs["Weight_V"], dtype=np.float32))

    nc = _get_nc()

    masks_h = [_host_masks(h) for h in (0, 1)]
    in_maps = []
    for c in range(8):
        b, h = c // 2, c % 2
        rows = np.concatenate(
            [np.arange((2 * t + h) * 128, (2 * t + h + 1) * 128) for t in range(NT)]
        )
        if USE_CC:
            xk_c = xk_full[b][h * SC : (h + 1) * SC]
            xv_c = xv_full[b][h * SC : (h + 1) * SC]
        else:
            xk_c, xv_c = xk_full[b], xv_full[b]
        in_maps.append(
            {
                "xkv": np.ascontiguousarray(
                    np.concatenate(
                        [
                            _prep_xt(xk_c),
                            _prep_xt(xv_c),
                            _prep_xt(xq_full[b][rows]),
                        ],
                        axis=1,
                    )
                ),
                "wq": wq,
                "wk": wk,
                "wv": wv,
                "masks": masks_h[h],
                "vtag": np.zeros((1, KERNEL_UID), np.float32),
            }
        )

    trace = bool(int(os.environ.get("KERNEL_TRACE", "0")))
    res = run_bass_kernel_spmd(
        nc, in_maps, core_ids=list(range(8)), trace=trace
    )
    if trace:
        _CACHE["last_results"] = res

    # avh[c] = [65, 512] AV^T for q-half c: rows 0:64 are V-weighted sums,
    # row 64 the softmax denominator.  Final divide + transpose on host.
    out_full = np.empty((B, S, E), dtype=np.float32)
    for c in range(8):
        b, h = c // 2, c % 2
        av = res.results[c]["avh"]                        # [65, 2, 512]
        for half in range(2):
            num = av[0:E, half, :]                        # [64, 512]
            den = av[E, half, :]                          # [512]
            blk = (num / den[None, :]).T                  # [512, 64] natural
            for j in range(4):
                t = 4 * half + j
                g = 2 * t + h
                out_full[b, g * 128 : (g + 1) * 128] = blk[
                    j * 128 : (j + 1) * 128
                ]
    return out_full
